# revision 1
# baseline (speedup 1.0000x reference)
"""Trainium2 Bass kernel for BeliefTreeMemory GNN message passing.

Strategy (8 NeuronCores, SPMD, one program):
  - Shard by tgt-node range: core c owns local nodes [0, 25000) = global
    [c*25000, (c+1)*25000).  Edges live on the core owning their tgt.
  - Edges sorted by tgt and grouped by 512-node tiles; per-tile counts
    padded to the max over cores (uniform compile-time structure).
  - h shard kept resident in SBUF, transposed [D, NSH_pad]:
      * tgt-reads  = gpsimd.ap_gather per 128-edge chunk (direct [D,E]).
      * GRU reads/writes the slab in place, tile by tile.
  - Aggregation: indicator matmul  aggT[D, 512] += m'^T @ S  accumulated
    in PSUM across a tile's chunks; S built on DVE by iota==tlocrel.
    recip[tgt] folded into m' (ACT scale); b2 folded into the xT copy.
  - GRU fully in transposed layout; has_msg mask folded as a rank-1
    K=1 matmul adding BIG*(1-has_msg) to the z-gate pre-activation.
  - Pass-1 h_src comes pre-gathered AND pre-transposed from the host
    (h0 is known); pass-2 h_src gathered from the AllGather output via
    per-chunk indirect DMA (128 rows/call).
  - One AllGather of row-major h1 shards between the passes.
"""

import sys
import numpy as np
import ml_dtypes

BF16 = ml_dtypes.bfloat16

sys.path.insert(0, "/opt/trn_rl_repo")

N_NODES = 200000
N_EDGES = 400000
D = 128
N_CORES = 8
N_PASSES = 2
TN = 512            # node-tile (and GRU chunk) size
FAKE_SILU = False   # decomposed silu (sim lacks Silu table)
BIG = 30.0


class _Cfg:
    def __init__(self, n_nodes=N_NODES, n_edges=N_EDGES):
        assert n_nodes % N_CORES == 0
        self.N = n_nodes
        self.E = n_edges
        self.NSH = n_nodes // N_CORES
        self.NSHP = ((self.NSH + TN - 1) // TN) * TN
        self.NTILES = self.NSHP // TN


def _rup(x, m):
    return ((x + m - 1) // m) * m


def _plan(cfg, inputs):
    """Host-side: per-core token layout + all swizzled input images."""
    N, NSH = cfg.N, cfg.NSH
    C = N_CORES
    src = np.asarray(inputs["src"]).astype(np.int64)
    tgt = np.asarray(inputs["tgt"]).astype(np.int64)
    etype = np.asarray(inputs["etype"]).astype(np.int64)
    cred = np.asarray(inputs["cred"], np.float32)
    h0 = np.asarray(inputs["h"], np.float32)
    E = src.shape[0]

    cnt = np.bincount(tgt, minlength=N).astype(np.int64)
    recip = (1.0 / np.maximum(cnt, 1)).astype(np.float32)

    core_of = tgt // NSH
    tloc = tgt - core_of * NSH
    tile_of = tloc // TN

    # per (core, tile) counts -> uniform padded sizes
    counts = np.zeros((C, cfg.NTILES), np.int64)
    for c in range(C):
        m = core_of == c
        counts[c] = np.bincount(tile_of[m], minlength=cfg.NTILES)
    P = np.array([_rup(max(int(counts[:, t].max()), 1), 128)
                  for t in range(cfg.NTILES)], np.int64)
    E_pad = int(P.sum())
    tile_of_chunk = np.repeat(np.arange(cfg.NTILES), P // 128)
    n_chunks = E_pad // 128

    recip_e = recip[tgt]
    cred_e = cred[src]

    per_core = []
    for c in range(C):
        SRC = np.zeros(E_pad, np.int64)
        TLOCREL = np.full(E_pad, -1.0, np.float32)
        TLOC = np.zeros(E_pad, np.int64)       # apg idx (tile-relative)
        REC = np.zeros(E_pad, np.float32)
        SRH = np.zeros((5, E_pad), np.float32)
        m = core_of == c
        eids = np.nonzero(m)[0]
        order = np.argsort(tloc[eids], kind="stable")
        eids = eids[order]
        et = tile_of[eids]
        off = 0
        for t in range(cfg.NTILES):
            ids = eids[et == t]
            nv = len(ids)
            sl = slice(off, off + nv)
            SRC[sl] = src[ids]
            TLOCREL[sl] = (tloc[ids] - t * TN).astype(np.float32)
            TLOC[sl] = tloc[ids] - t * TN
            REC[sl] = recip_e[ids]
            SRH[0, sl] = cred_e[ids]
            oh = np.eye(4, dtype=np.float32)[etype[ids]]
            SRH[1:5, sl] = oh.T
            off += int(P[t])
        assert off == E_pad

        # images
        srcg_img = SRC.reshape(n_chunks, 128).T.astype(np.int32)  # [128, nch]
        tlr_img = TLOCREL.reshape(n_chunks, 128).T.copy()         # [128, nch]
        rec_img = REC.reshape(n_chunks, 128).T.copy()             # [128, nch]
        # apg idx image: per chunk [128, 8]; row p, col s = u[s*16 + p%16]
        apg = np.zeros((128, 8 * n_chunks), np.int16)
        for ch in range(n_chunks):
            u = TLOC[ch * 128:(ch + 1) * 128]
            blk = u.reshape(8, 16).T.astype(np.int16)
            apg[:, 8 * ch:8 * (ch + 1)] = np.tile(blk, (8, 1))
        # pass-1 src feed, pre-transposed per chunk: [nch, 128 D, 128 E]
        feed = h0[SRC].reshape(n_chunks, 128, D).transpose(0, 2, 1)
        feed = np.ascontiguousarray(feed).astype(BF16)

        hshT = np.zeros((D, cfg.NSHP), np.float32)
        hshT[:, :NSH] = h0[c * NSH:(c + 1) * NSH].T
        nomsg = np.zeros((1, cfg.NSHP), np.float32)
        nomsg[0, :NSH] = BIG * (cnt[c * NSH:(c + 1) * NSH] == 0)
        nomsg[0, NSH:] = BIG

        per_core.append(dict(
            srcg=srcg_img, tlr=tlr_img, recg=rec_img, apg=apg,
            srhs=np.ascontiguousarray(SRH.astype(BF16)), feed=feed.reshape(-1),
            hsh0T=hshT, nomsg=nomsg.astype(BF16)))

    W1 = np.asarray(inputs["W1"], np.float32)
    ee = np.asarray(inputs["edge_emb"], np.float32)
    bih = np.asarray(inputs["bih"], np.float32)
    bhh = np.asarray(inputs["bhh"], np.float32)
    shared = dict(
        W1srcT=np.ascontiguousarray(W1[:, :D].T).astype(BF16),
        W1tgtT=np.ascontiguousarray(W1[:, D:2 * D].T).astype(BF16),
        W1staT=np.ascontiguousarray(np.concatenate(
            [W1[:, 2 * D + 64][None, :],
             ee @ W1[:, 2 * D:2 * D + 64].T], 0)).astype(BF16),
        b1col=np.asarray(inputs["b1"], np.float32)[:, None],
        W2T=np.ascontiguousarray(
            np.asarray(inputs["W2"], np.float32).T).astype(BF16),
        b2col=np.asarray(inputs["b2"], np.float32)[:, None],
        WihT=np.ascontiguousarray(
            np.asarray(inputs["Wih"], np.float32).T).astype(BF16),
        WhhT=np.ascontiguousarray(
            np.asarray(inputs["Whh"], np.float32).T).astype(BF16),
        brzcol=np.ascontiguousarray((bih + bhh)[:2 * D].reshape(2, D).T),
        bnhcol=bhh[2 * D:][:, None].copy(),
        bnicol=bih[2 * D:][:, None].copy(),
        ones1=np.ones((1, D), BF16),
        eye=np.eye(D, dtype=np.float32),
        iota=np.tile(np.arange(TN, dtype=np.float32), (128, 1)),
    )
    meta = dict(P=P, E_pad=E_pad, n_chunks=n_chunks,
                tile_of_chunk=tile_of_chunk)
    return meta, per_core, shared


def _build(cfg, meta):
    global FAKE_SILU
    from concourse import bacc, tile, mybir
    import concourse.bass as bass

    nc = bacc.Bacc("TRN2", target_bir_lowering=False, debug=False,
                   num_devices=N_CORES)
    f32, i32, i16 = mybir.dt.float32, mybir.dt.int32, mybir.dt.int16
    bf16 = mybir.dt.bfloat16
    AF = mybir.ActivationFunctionType
    NSH, NSHP = cfg.NSH, cfg.NSHP
    P = meta["P"]
    n_chunks = meta["n_chunks"]
    E_pad = meta["E_pad"]

    srcg = nc.dram_tensor("srcg", [128, n_chunks], i32, kind="ExternalInput")
    tlr = nc.dram_tensor("tlr", [128, n_chunks], f32, kind="ExternalInput")
    recg = nc.dram_tensor("recg", [128, n_chunks], f32, kind="ExternalInput")
    apg = nc.dram_tensor("apg", [128, 8 * n_chunks], i16,
                         kind="ExternalInput")
    srhs = nc.dram_tensor("srhs", [5, E_pad], bf16, kind="ExternalInput")
    feed = nc.dram_tensor("feed", [E_pad * 128], bf16, kind="ExternalInput")
    hsh0T = nc.dram_tensor("hsh0T", [D, NSHP], f32, kind="ExternalInput")
    nomsg = nc.dram_tensor("nomsg", [1, NSHP], bf16,
                           kind="ExternalInput")
    wnames = dict(W1srcT=[D, D], W1tgtT=[D, D], W1staT=[5, D],
                  b1col=[D, 1], W2T=[D, D], b2col=[D, 1],
                  WihT=[D, 3 * D], WhhT=[D, 3 * D], brzcol=[D, 2],
                  bnhcol=[D, 1], bnicol=[D, 1], ones1=[1, D], eye=[D, D],
                  iota=[128, TN])
    wbf = {"W1srcT", "W1tgtT", "W1staT", "W2T", "WihT", "WhhT", "ones1"}
    wt = {k: nc.dram_tensor(k, sh, bf16 if k in wbf else f32,
                            kind="ExternalInput")
          for k, sh in wnames.items()}
    h_out = nc.dram_tensor("h_out", [NSHP, D], f32, kind="ExternalOutput")
    h1rm = nc.dram_tensor("h1rm", [NSHP, D], f32)
    h1full = nc.dram_tensor("h1full", [cfg.N, D], f32, addr_space="Shared")

    with tile.TileContext(nc) as tc:
        with (
            tc.tile_pool(name="const", bufs=1) as cpool,
            tc.tile_pool(name="sfeed", bufs=5) as fpool,
            tc.tile_pool(name="work", bufs=6) as wpool,
            tc.tile_pool(name="gru", bufs=1) as upool,
            tc.tile_pool(name="pt", bufs=2, space="PSUM") as pt,
            tc.tile_pool(name="pg", bufs=4, space="PSUM") as pg,
            tc.tile_pool(name="pu", bufs=2, space="PSUM") as pu,
        ):
            w = {}
            for k, sh in wnames.items():
                w[k] = cpool.tile(sh, bf16 if k in wbf else f32,
                                  tag=k, name=f"w_{k}")
                nc.sync.dma_start(out=w[k][:, :], in_=wt[k][:, :])
            slab = cpool.tile([D, NSHP], f32, tag="slab")
            for t in range(cfg.NTILES):
                nc.sync.dma_start(out=slab[:, TN * t:TN * (t + 1)],
                                  in_=hsh0T[:, TN * t:TN * (t + 1)])
            srcg_sb = cpool.tile([128, n_chunks], i32, tag="srcg")
            nc.sync.dma_start(out=srcg_sb[:, :], in_=srcg[:, :])
            tlr_sb = cpool.tile([128, n_chunks], f32, tag="tlr")
            nc.sync.dma_start(out=tlr_sb[:, :], in_=tlr[:, :])
            rec_sb = cpool.tile([128, n_chunks], f32, tag="rec")
            nc.sync.dma_start(out=rec_sb[:, :], in_=recg[:, :])
            apg_sb = cpool.tile([128, 8 * n_chunks], i16, tag="apg")
            nc.sync.dma_start(out=apg_sb[:, :], in_=apg[:, :])

            for p in range(N_PASSES):
                ch0 = 0
                for t in range(cfg.NTILES):
                    tch = int(P[t]) // 128
                    aggT = pg.tile([128, TN], f32, tag="pg", name=f"agg{p}_{t}")
                    for b0 in range(0, tch, 4):
                        bw = min(4, tch - b0)
                        cb = ch0 + b0
                        sT = wpool.tile([128, 512], bf16, tag="sT")
                        tT = wpool.tile([128, 512], bf16, tag="tT")
                        tTf = fpool.tile([128, 512], f32, tag="tTf")
                        # S indicators first: no data deps, drain DVE early
                        Ss = []
                        for a in range(bw):
                            ch = cb + a
                            S = wpool.tile([128, TN], bf16, tag="S",
                                           name=f"S{p}_{ch}")
                            nc.vector.tensor_scalar(
                                out=S[:, :], in0=w["iota"][:, :],
                                scalar1=tlr_sb[:, ch:ch + 1],
                                scalar2=rec_sb[:, ch:ch + 1],
                                op0=mybir.AluOpType.is_equal,
                                op1=mybir.AluOpType.mult)
                            Ss.append(S)
                        # tgt gathers (gpsimd), then one block cast
                        for a in range(bw):
                            ch = cb + a
                            nc.gpsimd.ap_gather(
                                tTf[:, 128 * a:128 * (a + 1)],
                                slab[:, TN * t:TN * (t + 1)],
                                apg_sb[:, 8 * ch:8 * (ch + 1)],
                                channels=128, num_elems=TN, d=1, num_idxs=128)
                        nc.vector.tensor_copy(tT[:, :128 * bw],
                                              tTf[:, :128 * bw])
                        # src rows
                        if p == 0:
                            nc.sync.dma_start(
                                out=sT[:, :128 * bw]
                                .rearrange("p (a e) -> p a e", e=128),
                                in_=feed[cb * 128 * 128:
                                         (cb + bw) * 128 * 128]
                                .rearrange("(a p e) -> p a e", p=128, e=128))
                        else:
                            gsrs = []
                            for a in range(bw):
                                ch = cb + a
                                gsr = fpool.tile([128, 128], f32, tag="gsr",
                                                 name=f"gsr{p}_{ch}")
                                nc.gpsimd.indirect_dma_start(
                                    out=gsr[:, :], out_offset=None,
                                    in_=h1full[:, :],
                                    in_offset=bass.IndirectOffsetOnAxis(
                                        ap=srcg_sb[:, ch:ch + 1], axis=0))
                                gsrs.append(gsr)
                            ps = pt.tile([128, 512], f32, tag="pt",
                                         name=f"ps{p}_{cb}")
                            for a in range(bw):
                                nc.tensor.transpose(
                                    ps[:, 128 * a:128 * (a + 1)],
                                    gsrs[a][:, :], w["eye"][:, :])
                            nc.vector.tensor_copy(sT[:, :128 * bw],
                                                  ps[:, :128 * bw])
                        # layer 1
                        y1 = pg.tile([128, 512], f32, tag="pg",
                                     name=f"y1_{p}_{cb}")
                        nc.tensor.matmul(y1[:, :128 * bw], w["W1srcT"][:, :],
                                         sT[:, :128 * bw],
                                         start=True, stop=False)
                        nc.tensor.matmul(y1[:, :128 * bw], w["W1tgtT"][:, :],
                                         tT[:, :128 * bw],
                                         start=False, stop=False)
                        srh_t = fpool.tile([5, 512], bf16, tag="srh")
                        nc.sync.dma_start(
                            out=srh_t[:, :128 * bw],
                            in_=srhs[:, 128 * cb:128 * (cb + bw)])
                        nc.tensor.matmul(
                            y1[:, :128 * bw], w["W1staT"][:, :],
                            srh_t[:, :128 * bw],
                            start=False, stop=True)
                        y1s = wpool.tile([128, 512], bf16, tag="y1s")
                        if FAKE_SILU:
                            zb = wpool.tile([128, 512], f32, tag="zb")
                            nc.scalar.activation(zb[:, :128 * bw],
                                                 y1[:, :128 * bw],
                                                 AF.Identity,
                                                 bias=w["b1col"][:, 0:1])
                            sg = wpool.tile([128, 512], f32, tag="sg")
                            nc.scalar.activation(sg[:, :128 * bw],
                                                 y1[:, :128 * bw],
                                                 AF.Sigmoid,
                                                 bias=w["b1col"][:, 0:1])
                            nc.vector.tensor_mul(y1s[:, :128 * bw],
                                                 zb[:, :128 * bw],
                                                 sg[:, :128 * bw])
                        else:
                            nc.scalar.activation(y1s[:, :128 * bw],
                                                 y1[:, :128 * bw], AF.Silu,
                                                 bias=w["b1col"][:, 0:1])
                        # layer 2 + m' + aggregation
                        y2 = pt.tile([128, 512], f32, tag="pt",
                                     name=f"y2_{p}_{cb}")
                        mps = []
                        for a in range(bw):
                            nc.tensor.matmul(
                                y2[:, 128 * a:128 * (a + 1)],
                                y1s[:, 128 * a:128 * (a + 1)],
                                w["W2T"][:, :], start=True, stop=True)
                            mp = wpool.tile([128, 128], bf16, tag="mp",
                                            name=f"mp{p}_{cb + a}")
                            nc.vector.tensor_copy(
                                mp[:, :], y2[:, 128 * a:128 * (a + 1)])
                            mps.append(mp)
                        for a in range(bw):
                            nc.tensor.matmul(aggT[:, :], mps[a][:, :],
                                             Ss[a][:, :],
                                             start=(b0 == 0 and a == 0),
                                             stop=(b0 + 4 >= tch
                                                   and a == bw - 1))
                    ch0 += tch
                    # ---- GRU for this node tile (in T layout)
                    cl, chh = TN * t, TN * (t + 1)
                    xT = upool.tile([128, TN], bf16, tag="xT")
                    nc.scalar.activation(xT[:, :], aggT[:, :], AF.Identity,
                                         bias=w["b2col"][:, 0:1])
                    hTs = slab[:, cl:chh]
                    hTb = upool.tile([128, TN], bf16, tag="hTb")
                    nc.vector.tensor_copy(hTb[:, :], hTs)
                    pr = pu.tile([128, TN], f32, tag="pu", name=f"pr{p}_{t}")
                    pz = pu.tile([128, TN], f32, tag="pu", name=f"pz{p}_{t}")
                    nc.tensor.matmul(pr[:, :], w["WihT"][:, 0:D], xT[:, :],
                                     start=True, stop=False)
                    nc.tensor.matmul(pr[:, :], w["WhhT"][:, 0:D], hTb[:, :],
                                     start=False, stop=True)
                    nc.tensor.matmul(pz[:, :], w["WihT"][:, D:2 * D],
                                     xT[:, :], start=True, stop=False)
                    nc.tensor.matmul(pz[:, :], w["WhhT"][:, D:2 * D], hTb[:, :],
                                     start=False, stop=False)
                    nm_t = fpool.tile([1, TN], bf16, tag="nm")
                    nc.sync.dma_start(out=nm_t[:, :], in_=nomsg[:, cl:chh])
                    nc.tensor.matmul(pz[:, :], w["ones1"][:, :],
                                     nm_t[:, :], start=False, stop=True)
                    r_s = upool.tile([128, TN], f32, tag="r_s")
                    nc.scalar.activation(r_s[:, :], pr[:, :], AF.Sigmoid,
                                         bias=w["brzcol"][:, 0:1])
                    z_s = upool.tile([128, TN], f32, tag="z_s")
                    nc.scalar.activation(z_s[:, :], pz[:, :], AF.Sigmoid,
                                         bias=w["brzcol"][:, 1:2])
                    pni = pu.tile([128, TN], f32, tag="pu", name=f"pi{p}_{t}")
                    pnh = pu.tile([128, TN], f32, tag="pu", name=f"ph{p}_{t}")
                    nc.tensor.matmul(pni[:, :], w["WihT"][:, 2 * D:3 * D],
                                     xT[:, :], start=True, stop=True)
                    nc.tensor.matmul(pnh[:, :], w["WhhT"][:, 2 * D:3 * D],
                                     hTb[:, :], start=True, stop=True)
                    ghn = upool.tile([128, TN], f32, tag="ghn")
                    nc.scalar.activation(ghn[:, :], pnh[:, :], AF.Identity,
                                         bias=w["bnhcol"][:, 0:1])
                    t1 = upool.tile([128, TN], f32, tag="t1")
                    nc.vector.tensor_mul(t1[:, :], r_s[:, :], ghn[:, :])
                    t2 = upool.tile([128, TN], f32, tag="t2")
                    nc.vector.tensor_add(t2[:, :], pni[:, :], t1[:, :])
                    n_s = upool.tile([128, TN], f32, tag="n_s")
                    nc.scalar.activation(n_s[:, :], t2[:, :], AF.Tanh,
                                         bias=w["bnicol"][:, 0:1])
                    d_s = upool.tile([128, TN], f32, tag="d_s")
                    nc.vector.tensor_sub(d_s[:, :], hTs, n_s[:, :])
                    zd = upool.tile([128, TN], f32, tag="zd")
                    nc.vector.tensor_mul(zd[:, :], z_s[:, :], d_s[:, :])
                    hn = upool.tile([128, TN], f32, tag="hn")
                    nc.vector.tensor_add(hn[:, :], n_s[:, :], zd[:, :])
                    # write back into the resident slab (h for next pass)
                    nc.vector.tensor_copy(slab[:, cl:chh], hn[:, :])
                    # back-transpose to row-major for AllGather / output
                    hrows = upool.tile([128, TN], f32, tag="hrows")
                    pb = pt.tile([128, 512], f32, tag="pt", name=f"pb{p}_{t}")
                    for a in range(4):
                        nc.tensor.transpose(
                            pb[:, 128 * a:128 * (a + 1)],
                            hn[:, 128 * a:128 * (a + 1)], w["eye"][:, :])
                        nc.vector.tensor_copy(
                            hrows[:, 128 * a:128 * (a + 1)],
                            pb[:, 128 * a:128 * (a + 1)])
                    dst = h1rm if p == 0 else h_out
                    nc.sync.dma_start(
                        out=dst[cl:chh, :].rearrange("(a q) d -> q a d",
                                                     q=128),
                        in_=hrows[:, :].rearrange("q (a d) -> q a d", d=128))
                if p == 0:
                    nc.gpsimd.collective_compute(
                        "AllGather", mybir.AluOpType.bypass,
                        replica_groups=[list(range(N_CORES))],
                        ins=[h1rm[0:NSH, :]],
                        outs=[h1full[:, :]])
    nc.compile()
    return nc


def build_and_run(inputs, cfg=None, sim=False, trace=False, tmpdir=None):
    global FAKE_SILU
    cfg = cfg or _Cfg()
    meta, per_core, shared = _plan(cfg, inputs)
    FAKE_SILU = bool(sim)
    nc = _build(cfg, meta)
    maps = []
    for c in range(N_CORES):
        m = {k: np.ascontiguousarray(v) for k, v in per_core[c].items()}
        m.update({k: np.ascontiguousarray(v) for k, v in shared.items()})
        maps.append(m)
    if sim:
        from concourse.bass_interp import MultiCoreSim
        ms = MultiCoreSim(nc, num_cores=N_CORES, trace=False)
        for c in range(N_CORES):
            for k, v in maps[c].items():
                ms.cores[c].tensor(k)[:] = v
        ms.simulate(check_with_hw=False)
        shards = [np.array(ms.cores[c].tensor("h_out"))[:cfg.NSH]
                  for c in range(N_CORES)]
        return np.concatenate(shards, axis=0), None
    from concourse import bass_utils
    res = bass_utils.run_bass_kernel_spmd(
        nc, maps, list(range(N_CORES)), trace=trace, tmpdir=tmpdir)
    shards = [res.results[c]["h_out"][:cfg.NSH] for c in range(N_CORES)]
    return np.concatenate(shards, axis=0), res


def kernel(**inputs):
    out, _ = build_and_run(inputs)
    return out.astype(np.float32)



# revision 11
# speedup vs baseline: 1.5610x; 1.5610x over previous
"""Trainium2 Bass kernel for BeliefTreeMemory GNN message passing.

Strategy (8 NeuronCores, SPMD, one program):
  - Shard by tgt-node range: core c owns local nodes [0, 25000).  Edges
    live on the core owning their tgt, sorted by tgt, grouped per
    512-node tile, chunked into 128-edge chunks padded to the max over
    cores (uniform compile-time structure).
  - h shard resident in SBUF transposed [D, NSH_pad] (f32).
  - Messages in T layout: y1[mid, e] accumulated from const-stationary
    matmuls over pre-transposed src/tgt feeds; silu on ACT; per-chunk
    W2; aggregation via indicator matmul aggT += mp^T @ S with S
    HOST-BUILT (recip folded) and streamed from DRAM.
  - Pass-1 feeds (h0[src], h0[tgt]) pre-gathered AND pre-transposed on
    host.  Pass-2: src rows batch-gathered (4 chunks per indirect DMA)
    from the bf16 AllGather output, transposed on PE in bf16; tgt
    gathered from the f32 slab by gpsimd.ap_gather.
  - GRU fully in T layout, all-tanh form (sigmoid(x)=.5+.5tanh(x/2)) so
    every activation lives in the single `silu_and_others` table set.
    has_msg mask folded as rank-1 K=1 matmul adding BIG to the z gate.
  - h1 exchanged bf16: per-tile back-transpose (bf16) -> h1rm ->
    AllGather in 4 row-chunks issued as pass-1 tiles complete.
  - Final output stays T-layout f32; host transposes.
"""

import sys
import numpy as np
import ml_dtypes

BF16 = ml_dtypes.bfloat16

sys.path.insert(0, "/opt/trn_rl_repo")

N_NODES = 200000
N_EDGES = 400000
D = 128
N_CORES = 8
N_PASSES = 2
TN = 512            # node-tile size
NAG = 4             # AllGather row-chunks
FAKE_SILU = False   # decomposed silu (sim lacks Silu table)
BIG = 30.0


class _Cfg:
    def __init__(self, n_nodes=N_NODES, n_edges=N_EDGES):
        assert n_nodes % N_CORES == 0
        self.N = n_nodes
        self.E = n_edges
        self.NSH = n_nodes // N_CORES
        assert self.NSH % NAG == 0
        self.GR = self.NSH // NAG
        self.NSHP = ((self.NSH + TN - 1) // TN) * TN
        self.NTILES = self.NSHP // TN


def _rup(x, m):
    return ((x + m - 1) // m) * m


def _plan(cfg, inputs):
    """Host-side: per-core token layout + all swizzled input images."""
    N, NSH, GR = cfg.N, cfg.NSH, cfg.GR
    C = N_CORES
    src = np.asarray(inputs["src"]).astype(np.int64)
    tgt = np.asarray(inputs["tgt"]).astype(np.int64)
    etype = np.asarray(inputs["etype"]).astype(np.int64)
    cred = np.asarray(inputs["cred"], np.float32)
    h0 = np.asarray(inputs["h"], np.float32)

    cnt = np.bincount(tgt, minlength=N).astype(np.int64)
    recip = (1.0 / np.maximum(cnt, 1)).astype(np.float32)

    core_of = tgt // NSH
    tloc = tgt - core_of * NSH
    tile_of = tloc // TN

    counts = np.zeros((C, cfg.NTILES), np.int64)
    for c in range(C):
        m = core_of == c
        counts[c] = np.bincount(tile_of[m], minlength=cfg.NTILES)
    P = np.array([_rup(max(int(counts[:, t].max()), 1), 128)
                  for t in range(cfg.NTILES)], np.int64)
    E_pad = int(P.sum())
    n_chunks = E_pad // 128

    recip_e = recip[tgt]
    cred_e = cred[src]

    # h1full grouped layout: node (o, r) -> g*(C*GR) + o*GR + (r - g*GR)
    def h1idx(nodes):
        o = nodes // NSH
        r = nodes - o * NSH
        g = r // GR
        return g * (C * GR) + o * GR + (r - g * GR)

    per_core = []
    for c in range(C):
        SRC = np.zeros(E_pad, np.int64)
        TGT = np.zeros(E_pad, np.int64)
        TLOCREL = np.full(E_pad, -1, np.int64)   # within-tile node, -1 pad
        TLOC = np.zeros(E_pad, np.int64)         # apg idx (tile-relative)
        REC = np.zeros(E_pad, np.float32)
        SRH = np.zeros((5, E_pad), np.float32)
        m = core_of == c
        eids = np.nonzero(m)[0]
        order = np.argsort(tloc[eids], kind="stable")
        eids = eids[order]
        et = tile_of[eids]
        off = 0
        for t in range(cfg.NTILES):
            ids = eids[et == t]
            nv = len(ids)
            sl = slice(off, off + nv)
            SRC[sl] = src[ids]
            TGT[sl] = tgt[ids]
            TLOCREL[sl] = tloc[ids] - t * TN
            TLOC[sl] = tloc[ids] - t * TN
            REC[sl] = recip_e[ids]
            SRH[0, sl] = cred_e[ids]
            oh = np.eye(4, dtype=np.float32)[etype[ids]]
            SRH[1:5, sl] = oh.T
            off += int(P[t])
        assert off == E_pad

        # pass-2 src gather indices into grouped h1full: [128, n_chunks]
        srcg_img = h1idx(SRC).reshape(n_chunks, 128).T.astype(np.int32)
        # apg idx image: per chunk [128, 8]; row p, col s = u[s*16 + p%16]
        apg = np.zeros((128, 8 * n_chunks), np.int16)
        for ch in range(n_chunks):
            u = TLOC[ch * 128:(ch + 1) * 128]
            blk = u.reshape(8, 16).T.astype(np.int16)
            apg[:, 8 * ch:8 * (ch + 1)] = np.tile(blk, (8, 1))
        # host-built S: per chunk [128 e, 512 nodes] bf16, recip folded
        S_img = np.zeros((128, TN * n_chunks), np.float32)
        rows = np.arange(128)
        for ch in range(n_chunks):
            tl = TLOCREL[ch * 128:(ch + 1) * 128]
            rc = REC[ch * 128:(ch + 1) * 128]
            valid = tl >= 0
            S_img[rows[valid], TN * ch + tl[valid]] = rc[valid]
        # pass-1 feeds, pre-transposed per chunk: [nch, 128 D, 128 E]
        feed_s = h0[SRC].reshape(n_chunks, 128, D).transpose(0, 2, 1)
        feed_t = h0[TGT].reshape(n_chunks, 128, D).transpose(0, 2, 1)

        hshT = np.zeros((D, cfg.NSHP), np.float32)
        hshT[:, :NSH] = h0[c * NSH:(c + 1) * NSH].T
        nomsg = np.zeros((1, cfg.NSHP), np.float32)
        nomsg[0, :NSH] = BIG * (cnt[c * NSH:(c + 1) * NSH] == 0)
        nomsg[0, NSH:] = BIG

        per_core.append(dict(
            srcg=srcg_img, apg=apg,
            S=np.ascontiguousarray(S_img.astype(BF16)),
            srhs=np.ascontiguousarray(SRH.astype(BF16)),
            feeds=np.ascontiguousarray(feed_s.astype(BF16)).reshape(-1),
            feedt=np.ascontiguousarray(feed_t.astype(BF16)).reshape(-1),
            hsh0T=hshT, nomsg=nomsg.astype(BF16)))

    W1 = np.asarray(inputs["W1"], np.float32)
    ee = np.asarray(inputs["edge_emb"], np.float32)
    bih = np.asarray(inputs["bih"], np.float32)
    bhh = np.asarray(inputs["bhh"], np.float32)
    Wih = np.asarray(inputs["Wih"], np.float32)
    WihT = Wih.T.copy()          # [D, 3D]
    WihT[:, 2 * D:] *= 2.0       # n-gate doubled (tanh half-angle form)
    shared = dict(
        W1srcT=np.ascontiguousarray(W1[:, :D].T).astype(BF16),
        W1tgtT=np.ascontiguousarray(W1[:, D:2 * D].T).astype(BF16),
        W1staT=np.ascontiguousarray(np.concatenate(
            [W1[:, 2 * D + 64][None, :],
             ee @ W1[:, 2 * D:2 * D + 64].T], 0)).astype(BF16),
        b1col=np.asarray(inputs["b1"], np.float32)[:, None],
        W2T=np.ascontiguousarray(
            np.asarray(inputs["W2"], np.float32).T).astype(BF16),
        b2col=np.asarray(inputs["b2"], np.float32)[:, None],
        WihT=np.ascontiguousarray(WihT).astype(BF16),
        WhhT=np.ascontiguousarray(
            np.asarray(inputs["Whh"], np.float32).T).astype(BF16),
        brzcol=np.ascontiguousarray(
            0.5 * (bih + bhh)[:2 * D].reshape(2, D).T),
        bnhcol=bhh[2 * D:][:, None].copy(),
        bnicol=bih[2 * D:][:, None].copy(),
        ones1=np.ones((1, D), BF16),
        eye=np.eye(D, dtype=BF16),
    )
    meta = dict(P=P, E_pad=E_pad, n_chunks=n_chunks)
    return meta, per_core, shared


def _build(cfg, meta):
    global FAKE_SILU
    from concourse import bacc, tile, mybir
    import concourse.bass as bass

    nc = bacc.Bacc("TRN2", target_bir_lowering=False, debug=False,
                   num_devices=N_CORES)
    f32, i32, i16 = mybir.dt.float32, mybir.dt.int32, mybir.dt.int16
    bf16 = mybir.dt.bfloat16
    AF = mybir.ActivationFunctionType
    NSH, NSHP, GR = cfg.NSH, cfg.NSHP, cfg.GR
    P = meta["P"]
    n_chunks = meta["n_chunks"]
    E_pad = meta["E_pad"]
    TCH_MAX = int(max(P)) // 128

    srcg = nc.dram_tensor("srcg", [128, n_chunks], i32, kind="ExternalInput")
    apg = nc.dram_tensor("apg", [128, 8 * n_chunks], i16,
                         kind="ExternalInput")
    S_d = nc.dram_tensor("S", [128, TN * n_chunks], bf16,
                         kind="ExternalInput")
    srhs = nc.dram_tensor("srhs", [5, E_pad], bf16, kind="ExternalInput")
    feeds = nc.dram_tensor("feeds", [E_pad * 128], bf16,
                           kind="ExternalInput")
    feedt = nc.dram_tensor("feedt", [E_pad * 128], bf16,
                           kind="ExternalInput")
    hsh0T = nc.dram_tensor("hsh0T", [D, NSHP], f32, kind="ExternalInput")
    nomsg = nc.dram_tensor("nomsg", [1, NSHP], bf16, kind="ExternalInput")
    wnames = dict(W1srcT=[D, D], W1tgtT=[D, D], W1staT=[5, D],
                  b1col=[D, 1], W2T=[D, D], b2col=[D, 1],
                  WihT=[D, 3 * D], WhhT=[D, 3 * D], brzcol=[D, 2],
                  bnhcol=[D, 1], bnicol=[D, 1], ones1=[1, D], eye=[D, D])
    wbf = {"W1srcT", "W1tgtT", "W1staT", "W2T", "WihT", "WhhT", "ones1",
           "eye"}
    wt = {k: nc.dram_tensor(k, sh, bf16 if k in wbf else f32,
                            kind="ExternalInput")
          for k, sh in wnames.items()}
    h_outT = nc.dram_tensor("h_outT", [D, NSHP], f32, kind="ExternalOutput")
    h1rm = nc.dram_tensor("h1rm", [NSHP, D], bf16)
    h1full = nc.dram_tensor("h1full", [cfg.N, D], bf16, addr_space="Shared")

    with tile.TileContext(nc) as tc:
        with (
            tc.tile_pool(name="const", bufs=1) as cpool,
            tc.tile_pool(name="stream", bufs=3) as spool,
            tc.tile_pool(name="sfeed", bufs=4) as fpool,
            tc.tile_pool(name="work", bufs=4) as wpool,
            tc.tile_pool(name="gru", bufs=2) as upool,
            tc.tile_pool(name="pt", bufs=2, space="PSUM") as pt,
            tc.tile_pool(name="pg", bufs=4, space="PSUM") as pg,
            tc.tile_pool(name="pu", bufs=2, space="PSUM") as pu,
        ):
            w = {}
            for k, sh in wnames.items():
                w[k] = cpool.tile(sh, bf16 if k in wbf else f32,
                                  tag=k, name=f"w_{k}")
                nc.sync.dma_start(out=w[k][:, :], in_=wt[k][:, :])
            slab = cpool.tile([D, NSHP], f32, tag="slab")
            for t in range(cfg.NTILES):
                nc.sync.dma_start(out=slab[:, TN * t:TN * (t + 1)],
                                  in_=hsh0T[:, TN * t:TN * (t + 1)])
            srcg_sb = cpool.tile([128, n_chunks], i32, tag="srcg")
            nc.sync.dma_start(out=srcg_sb[:, :], in_=srcg[:, :])
            apg_sb = cpool.tile([128, 8 * n_chunks], i16, tag="apg")
            nc.sync.dma_start(out=apg_sb[:, :], in_=apg[:, :])

            for p in range(N_PASSES):
                ch0 = 0
                next_g = 0
                for t in range(cfg.NTILES):
                    tch = int(P[t]) // 128
                    # per-tile streamed structure (ACT HWDGE queue)
                    srh_t = spool.tile([5, 128 * TCH_MAX], bf16, tag="srh")
                    nc.scalar.dma_start(
                        out=srh_t[:, :128 * tch],
                        in_=srhs[:, 128 * ch0:128 * (ch0 + tch)])
                    nm_t = spool.tile([1, TN], bf16, tag="nm")
                    nc.scalar.dma_start(out=nm_t[:, :],
                                        in_=nomsg[:, TN * t:TN * (t + 1)])
                    aggT = pg.tile([128, TN], f32, tag="pg",
                                   name=f"agg{p}_{t}")
                    for b0 in range(0, tch, 4):
                        bw = min(4, tch - b0)
                        cb = ch0 + b0
                        S_t = spool.tile([128, TN * 4], bf16, tag="S")
                        nc.scalar.dma_start(
                            out=S_t[:, :TN * bw],
                            in_=S_d[:, TN * cb:TN * (cb + bw)])
                        sT = fpool.tile([128, 512], bf16, tag="sT")
                        tT = fpool.tile([128, 512], bf16, tag="tT")
                        if p == 0:
                            nc.sync.dma_start(
                                out=sT[:, :128 * bw]
                                .rearrange("p (a e) -> p a e", e=128),
                                in_=feeds[cb * 128 * 128:
                                          (cb + bw) * 128 * 128]
                                .rearrange("(a p e) -> p a e", p=128, e=128))
                            nc.sync.dma_start(
                                out=tT[:, :128 * bw]
                                .rearrange("p (a e) -> p a e", e=128),
                                in_=feedt[cb * 128 * 128:
                                          (cb + bw) * 128 * 128]
                                .rearrange("(a p e) -> p a e", p=128, e=128))
                        else:
                            # src: per-chunk indirect gathers + bf16 T
                            gsr = fpool.tile([128, 512], bf16, tag="gsr")
                            for a in range(bw):
                                nc.gpsimd.indirect_dma_start(
                                    out=gsr[:, 128 * a:128 * (a + 1)],
                                    out_offset=None,
                                    in_=h1full[:, :],
                                    in_offset=bass.IndirectOffsetOnAxis(
                                        ap=srcg_sb[:, cb + a:cb + a + 1],
                                        axis=0))
                            ps = pt.tile([128, 512], bf16, tag="ptb",
                                         name=f"ps{p}_{cb}")
                            for a in range(bw):
                                nc.tensor.transpose(
                                    ps[:, 128 * a:128 * (a + 1)],
                                    gsr[:, 128 * a:128 * (a + 1)],
                                    w["eye"][:, :])
                            nc.vector.tensor_copy(sT[:, :128 * bw],
                                                  ps[:, :128 * bw])
                            # tgt: gpsimd gather from f32 slab
                            tTf = fpool.tile([128, 512], f32, tag="tTf")
                            for a in range(bw):
                                ch = cb + a
                                nc.gpsimd.ap_gather(
                                    tTf[:, 128 * a:128 * (a + 1)],
                                    slab[:, TN * t:TN * (t + 1)],
                                    apg_sb[:, 8 * ch:8 * (ch + 1)],
                                    channels=128, num_elems=TN, d=1,
                                    num_idxs=128)
                            nc.vector.tensor_copy(tT[:, :128 * bw],
                                                  tTf[:, :128 * bw])
                        # layer 1 (const stationaries)
                        y1 = pg.tile([128, 512], f32, tag="pg",
                                     name=f"y1_{p}_{cb}")
                        nc.tensor.matmul(y1[:, :128 * bw], w["W1srcT"][:, :],
                                         sT[:, :128 * bw],
                                         start=True, stop=False)
                        nc.tensor.matmul(y1[:, :128 * bw], w["W1tgtT"][:, :],
                                         tT[:, :128 * bw],
                                         start=False, stop=False)
                        nc.tensor.matmul(
                            y1[:, :128 * bw], w["W1staT"][:, :],
                            srh_t[:, 128 * b0:128 * (b0 + bw)],
                            start=False, stop=True)
                        y1s = wpool.tile([128, 512], bf16, tag="y1s")
                        if FAKE_SILU:
                            zb = wpool.tile([128, 512], f32, tag="zb")
                            nc.scalar.activation(zb[:, :128 * bw],
                                                 y1[:, :128 * bw],
                                                 AF.Identity,
                                                 bias=w["b1col"][:, 0:1])
                            sg = wpool.tile([128, 512], f32, tag="sg")
                            nc.scalar.activation(sg[:, :128 * bw],
                                                 y1[:, :128 * bw],
                                                 AF.Sigmoid,
                                                 bias=w["b1col"][:, 0:1])
                            nc.vector.tensor_mul(y1s[:, :128 * bw],
                                                 zb[:, :128 * bw],
                                                 sg[:, :128 * bw])
                        else:
                            nc.scalar.activation(y1s[:, :128 * bw],
                                                 y1[:, :128 * bw], AF.Silu,
                                                 bias=w["b1col"][:, 0:1])
                        # layer 2 per chunk; one grouped cast
                        y2 = pg.tile([128, 512], f32, tag="pg",
                                     name=f"y2_{p}_{cb}")
                        for a in range(bw):
                            nc.tensor.matmul(
                                y2[:, 128 * a:128 * (a + 1)],
                                y1s[:, 128 * a:128 * (a + 1)],
                                w["W2T"][:, :], start=True, stop=True)
                        mp = wpool.tile([128, 512], bf16, tag="mp")
                        nc.vector.tensor_copy(mp[:, :128 * bw],
                                              y2[:, :128 * bw])
                        for a in range(bw):
                            nc.tensor.matmul(
                                aggT[:, :], mp[:, 128 * a:128 * (a + 1)],
                                S_t[:, TN * a:TN * (a + 1)],
                                start=(b0 == 0 and a == 0),
                                stop=(b0 + 4 >= tch and a == bw - 1))
                    ch0 += tch
                    # ---- GRU for this node tile (T layout, all-tanh)
                    cl, chh = TN * t, TN * (t + 1)
                    xT = upool.tile([128, TN], bf16, tag="xT")
                    nc.scalar.activation(xT[:, :], aggT[:, :], AF.Identity,
                                         bias=w["b2col"][:, 0:1])
                    hTb = upool.tile([128, TN], bf16, tag="hTb")
                    nc.scalar.activation(hTb[:, :], slab[:, cl:chh],
                                         AF.Identity)
                    pr = pu.tile([128, TN], f32, tag="pu", name=f"pr{p}_{t}")
                    pz = pu.tile([128, TN], f32, tag="pu", name=f"pz{p}_{t}")
                    nc.tensor.matmul(pr[:, :], w["WihT"][:, 0:D], xT[:, :],
                                     start=True, stop=False)
                    nc.tensor.matmul(pr[:, :], w["WhhT"][:, 0:D], hTb[:, :],
                                     start=False, stop=True)
                    nc.tensor.matmul(pz[:, :], w["WihT"][:, D:2 * D],
                                     xT[:, :], start=True, stop=False)
                    nc.tensor.matmul(pz[:, :], w["WhhT"][:, D:2 * D],
                                     hTb[:, :], start=False, stop=False)
                    nc.tensor.matmul(pz[:, :], w["ones1"][:, :],
                                     nm_t[:, :], start=False, stop=True)
                    t_r = upool.tile([128, TN], bf16, tag="t_r")
                    nc.scalar.activation(t_r[:, :], pr[:, :], AF.Tanh,
                                         bias=w["brzcol"][:, 0:1], scale=0.5)
                    t_z = upool.tile([128, TN], bf16, tag="t_z")
                    nc.scalar.activation(t_z[:, :], pz[:, :], AF.Tanh,
                                         bias=w["brzcol"][:, 1:2], scale=0.5)
                    pni = pu.tile([128, TN], f32, tag="pu", name=f"pi{p}_{t}")
                    pnh = pu.tile([128, TN], f32, tag="pu", name=f"ph{p}_{t}")
                    nc.tensor.matmul(pni[:, :], w["WihT"][:, 2 * D:3 * D],
                                     xT[:, :], start=True, stop=True)
                    nc.tensor.matmul(pnh[:, :], w["WhhT"][:, 2 * D:3 * D],
                                     hTb[:, :], start=True, stop=True)
                    ghn = upool.tile([128, TN], bf16, tag="ghn")
                    nc.vector.tensor_scalar(
                        out=ghn[:, :], in0=pnh[:, :],
                        scalar1=w["bnhcol"][:, 0:1], scalar2=None,
                        op0=mybir.AluOpType.add)
                    m1 = upool.tile([128, TN], bf16, tag="m1")
                    nc.vector.tensor_mul(m1[:, :], t_r[:, :], ghn[:, :])
                    m2 = upool.tile([128, TN], bf16, tag="m2")
                    nc.vector.tensor_add(m2[:, :], m1[:, :], ghn[:, :])
                    m4 = upool.tile([128, TN], bf16, tag="m4")
                    nc.vector.tensor_add(m4[:, :], m2[:, :], pni[:, :])
                    n_s = upool.tile([128, TN], bf16, tag="n_s")
                    nc.scalar.activation(n_s[:, :], m4[:, :], AF.Tanh,
                                         bias=w["bnicol"][:, 0:1], scale=0.5)
                    d_s = upool.tile([128, TN], bf16, tag="d_s")
                    nc.vector.tensor_sub(d_s[:, :], hTb[:, :], n_s[:, :])
                    e_s = upool.tile([128, TN], bf16, tag="e_s")
                    nc.vector.tensor_mul(e_s[:, :], t_z[:, :], d_s[:, :])
                    f_s = upool.tile([128, TN], bf16, tag="f_s")
                    nc.vector.tensor_add(f_s[:, :], n_s[:, :], hTb[:, :])
                    g_s = upool.tile([128, TN], bf16, tag="g_s")
                    nc.vector.tensor_add(g_s[:, :], e_s[:, :], f_s[:, :])
                    nc.vector.tensor_scalar(
                        out=slab[:, cl:chh], in0=g_s[:, :],
                        scalar1=0.5, scalar2=None,
                        op0=mybir.AluOpType.mult)
                    if p == 0:
                        # bf16 rows for the AllGather
                        hn_bf = upool.tile([128, TN], bf16, tag="hn_bf")
                        nc.vector.tensor_scalar(
                            out=hn_bf[:, :], in0=g_s[:, :],
                            scalar1=0.5, scalar2=None,
                            op0=mybir.AluOpType.mult)
                        pb = pt.tile([128, 512], bf16, tag="ptb",
                                     name=f"pb{p}_{t}")
                        for a in range(4):
                            nc.tensor.transpose(
                                pb[:, 128 * a:128 * (a + 1)],
                                hn_bf[:, 128 * a:128 * (a + 1)],
                                w["eye"][:, :])
                        hrows = upool.tile([128, TN], bf16, tag="hrows")
                        nc.vector.tensor_copy(hrows[:, :], pb[:, :])
                        nc.sync.dma_start(
                            out=h1rm[cl:chh, :].rearrange(
                                "(a q) d -> q a d", q=128),
                            in_=hrows[:, :].rearrange(
                                "q (a d) -> q a d", d=128))
                        while (next_g < NAG
                               and (t + 1) * TN >= (next_g + 1) * GR):
                            g = next_g
                            nc.gpsimd.collective_compute(
                                "AllGather", mybir.AluOpType.bypass,
                                replica_groups=[list(range(N_CORES))],
                                ins=[h1rm[g * GR:(g + 1) * GR, :]],
                                outs=[h1full[g * N_CORES * GR:
                                             (g + 1) * N_CORES * GR, :]])
                            next_g += 1
                    else:
                        nc.sync.dma_start(out=h_outT[:, cl:chh],
                                          in_=slab[:, cl:chh])
    nc.compile()
    return nc


def build_and_run(inputs, cfg=None, sim=False, trace=False, tmpdir=None):
    global FAKE_SILU
    cfg = cfg or _Cfg()
    meta, per_core, shared = _plan(cfg, inputs)
    FAKE_SILU = bool(sim)
    nc = _build(cfg, meta)
    maps = []
    for c in range(N_CORES):
        m = {k: np.ascontiguousarray(v) for k, v in per_core[c].items()}
        m.update({k: np.ascontiguousarray(v) for k, v in shared.items()})
        maps.append(m)
    if sim:
        from concourse.bass_interp import MultiCoreSim
        ms = MultiCoreSim(nc, num_cores=N_CORES, trace=False)
        for c in range(N_CORES):
            for k, v in maps[c].items():
                ms.cores[c].tensor(k)[:] = v
        ms.simulate(check_with_hw=False)
        shards = [np.array(ms.cores[c].tensor("h_outT"))[:, :cfg.NSH].T
                  for c in range(N_CORES)]
        return np.concatenate(shards, axis=0), None
    from concourse import bass_utils
    res = bass_utils.run_bass_kernel_spmd(
        nc, maps, list(range(N_CORES)), trace=trace, tmpdir=tmpdir)
    shards = [res.results[c]["h_outT"][:, :cfg.NSH].T for c in range(N_CORES)]
    return np.concatenate(shards, axis=0), res


def kernel(**inputs):
    out, _ = build_and_run(inputs)
    return np.ascontiguousarray(out).astype(np.float32)


# revision 21
# speedup vs baseline: 1.6634x; 1.0656x over previous
"""Trainium2 Bass kernel for BeliefTreeMemory GNN message passing.

Strategy (8 NeuronCores, SPMD, one program):
  - Shard by tgt-node range: core c owns local nodes [0, 25000).  Edges
    live on the core owning their tgt, sorted by tgt, grouped per
    512-node tile, chunked into 128-edge chunks padded to the max over
    cores (uniform compile-time structure).
  - h shard resident in SBUF transposed [D, NSH_pad] (f32).
  - Messages in T layout: y1[mid, e] accumulated from const-stationary
    matmuls over pre-transposed src/tgt feeds; silu on ACT; per-chunk
    W2; aggregation via indicator matmul aggT += mp^T @ S with S
    HOST-BUILT (recip folded) and streamed from DRAM.
  - Pass-1 feeds (h0[src], h0[tgt]) pre-gathered AND pre-transposed on
    host.  Pass-2: src rows batch-gathered (4 chunks per indirect DMA)
    from the bf16 AllGather output, transposed on PE in bf16; tgt
    gathered from the f32 slab by gpsimd.ap_gather.
  - GRU fully in T layout, all-tanh form (sigmoid(x)=.5+.5tanh(x/2)) so
    every activation lives in the single `silu_and_others` table set.
    has_msg mask folded as rank-1 K=1 matmul adding BIG to the z gate.
  - h1 exchanged bf16: per-tile back-transpose (bf16) -> h1rm ->
    AllGather in 4 row-chunks issued as pass-1 tiles complete.
  - Final output stays T-layout f32; host transposes.
"""

import sys
import numpy as np
import ml_dtypes

BF16 = ml_dtypes.bfloat16

sys.path.insert(0, "/opt/trn_rl_repo")

N_NODES = 200000
N_EDGES = 400000
D = 128
N_CORES = 8
N_PASSES = 2
TN = 512            # node-tile size
# AllGather row-chunk fractions: big early (overlap pass-1), small last
AG_FRACS = [0.30, 0.25, 0.18, 0.14, 0.08, 0.05]
FAKE_SILU = False   # decomposed silu (sim lacks Silu table)
BIG = 30.0


class _Cfg:
    def __init__(self, n_nodes=N_NODES, n_edges=N_EDGES):
        assert n_nodes % N_CORES == 0
        self.N = n_nodes
        self.E = n_edges
        self.NSH = n_nodes // N_CORES
        self.NSHP = ((self.NSH + TN - 1) // TN) * TN
        self.NTILES = self.NSHP // TN
        # AllGather group row boundaries within a shard
        bnd = [0]
        acc = 0.0
        for f in AG_FRACS[:-1]:
            acc += f
            bnd.append(min(int(round(acc * self.NSH)), self.NSH))
        bnd.append(self.NSH)
        self.AGB = bnd                      # len NAG+1, ascending
        self.NAG = len(bnd) - 1


def _rup(x, m):
    return ((x + m - 1) // m) * m


def _plan(cfg, inputs):
    """Host-side: per-core token layout + all swizzled input images."""
    N, NSH = cfg.N, cfg.NSH
    C = N_CORES
    AGB = np.asarray(cfg.AGB, np.int64)
    # start offset of group g in the grouped h1full layout
    AGOFF = np.concatenate([[0], np.cumsum((AGB[1:] - AGB[:-1]) * C)])
    src = np.asarray(inputs["src"]).astype(np.int64)
    tgt = np.asarray(inputs["tgt"]).astype(np.int64)
    etype = np.asarray(inputs["etype"]).astype(np.int64)
    cred = np.asarray(inputs["cred"], np.float32)
    h0 = np.asarray(inputs["h"], np.float32)

    cnt = np.bincount(tgt, minlength=N).astype(np.int64)
    recip = (1.0 / np.maximum(cnt, 1)).astype(np.float32)

    core_of = tgt // NSH
    tloc = tgt - core_of * NSH
    tile_of = tloc // TN

    counts = np.zeros((C, cfg.NTILES), np.int64)
    for c in range(C):
        m = core_of == c
        counts[c] = np.bincount(tile_of[m], minlength=cfg.NTILES)
    P = np.array([_rup(max(int(counts[:, t].max()), 1), 128)
                  for t in range(cfg.NTILES)], np.int64)
    E_pad = int(P.sum())
    n_chunks = E_pad // 128

    recip_e = recip[tgt]
    cred_e = cred[src]

    # h1full grouped layout: node (o, r in group g) ->
    #   AGOFF[g] + o*rows_g + (r - AGB[g])
    def h1idx(nodes):
        o = nodes // NSH
        r = nodes - o * NSH
        g = np.searchsorted(AGB, r, side="right") - 1
        rows_g = AGB[g + 1] - AGB[g]
        return AGOFF[g] + o * rows_g + (r - AGB[g])

    per_core = []
    for c in range(C):
        SRC = np.zeros(E_pad, np.int64)
        TGT = np.zeros(E_pad, np.int64)
        TLOCREL = np.full(E_pad, -1, np.int64)   # within-tile node, -1 pad
        TLOC = np.zeros(E_pad, np.int64)         # apg idx (tile-relative)
        REC = np.zeros(E_pad, np.float32)
        SRH = np.zeros((5, E_pad), np.float32)
        m = core_of == c
        eids = np.nonzero(m)[0]
        order = np.argsort(tloc[eids], kind="stable")
        eids = eids[order]
        et = tile_of[eids]
        off = 0
        for t in range(cfg.NTILES):
            ids = eids[et == t]
            nv = len(ids)
            sl = slice(off, off + nv)
            SRC[sl] = src[ids]
            TGT[sl] = tgt[ids]
            TLOCREL[sl] = tloc[ids] - t * TN
            TLOC[sl] = tloc[ids] - t * TN
            REC[sl] = recip_e[ids]
            SRH[0, sl] = cred_e[ids]
            oh = np.eye(4, dtype=np.float32)[etype[ids]]
            SRH[1:5, sl] = oh.T
            off += int(P[t])
        assert off == E_pad

        # pass-2 src gather indices into grouped h1full: [128, n_chunks]
        srcg_img = h1idx(SRC).reshape(n_chunks, 128).T.astype(np.int32)
        # apg idx image: per chunk [128, 8]; row p, col s = u[s*16 + p%16]
        apg = np.zeros((128, 8 * n_chunks), np.int16)
        for ch in range(n_chunks):
            u = TLOC[ch * 128:(ch + 1) * 128]
            blk = u.reshape(8, 16).T.astype(np.int16)
            apg[:, 8 * ch:8 * (ch + 1)] = np.tile(blk, (8, 1))
        # host-built S: per chunk [128 e, 512 nodes] bf16, recip folded
        S_img = np.zeros((128, TN * n_chunks), np.float32)
        rows = np.arange(128)
        for ch in range(n_chunks):
            tl = TLOCREL[ch * 128:(ch + 1) * 128]
            rc = REC[ch * 128:(ch + 1) * 128]
            valid = tl >= 0
            S_img[rows[valid], TN * ch + tl[valid]] = rc[valid]
        # pass-1 feeds, pre-transposed per chunk: [nch, 128 D, 128 E]
        feed_s = h0[SRC].reshape(n_chunks, 128, D).transpose(0, 2, 1)
        feed_t = h0[TGT].reshape(n_chunks, 128, D).transpose(0, 2, 1)

        hshT = np.zeros((D, cfg.NSHP), np.float32)
        hshT[:, :NSH] = h0[c * NSH:(c + 1) * NSH].T
        nomsg = np.zeros((1, cfg.NSHP), np.float32)
        nomsg[0, :NSH] = BIG * (cnt[c * NSH:(c + 1) * NSH] == 0)
        nomsg[0, NSH:] = BIG

        per_core.append(dict(
            srcg=srcg_img, apg=apg,
            S=np.ascontiguousarray(S_img.astype(BF16)),
            srhs=np.ascontiguousarray(SRH.astype(BF16)),
            feeds=np.ascontiguousarray(feed_s.astype(BF16)).reshape(-1),
            feedt=np.ascontiguousarray(feed_t.astype(BF16)).reshape(-1),
            hsh0T=hshT, nomsg=nomsg.astype(BF16)))

    W1 = np.asarray(inputs["W1"], np.float32)
    ee = np.asarray(inputs["edge_emb"], np.float32)
    bih = np.asarray(inputs["bih"], np.float32)
    bhh = np.asarray(inputs["bhh"], np.float32)
    Wih = np.asarray(inputs["Wih"], np.float32)
    WihT = Wih.T.copy()          # [D, 3D]
    WihT[:, 2 * D:] *= 2.0       # n-gate doubled (tanh half-angle form)
    shared = dict(
        W1srcT=np.ascontiguousarray(W1[:, :D].T).astype(BF16),
        W1tgtT=np.ascontiguousarray(W1[:, D:2 * D].T).astype(BF16),
        W1staT=np.ascontiguousarray(np.concatenate(
            [W1[:, 2 * D + 64][None, :],
             ee @ W1[:, 2 * D:2 * D + 64].T], 0)).astype(BF16),
        b1col=np.asarray(inputs["b1"], np.float32)[:, None],
        W2T=np.ascontiguousarray(
            np.asarray(inputs["W2"], np.float32).T).astype(BF16),
        b2col=np.asarray(inputs["b2"], np.float32)[:, None],
        WihT=np.ascontiguousarray(WihT).astype(BF16),
        WhhT=np.ascontiguousarray(
            np.asarray(inputs["Whh"], np.float32).T).astype(BF16),
        brzcol=np.ascontiguousarray(
            0.5 * (bih + bhh)[:2 * D].reshape(2, D).T),
        bnhcol=bhh[2 * D:][:, None].copy(),
        bnicol=bih[2 * D:][:, None].copy(),
        ones1=np.ones((1, D), BF16),
        eye=np.eye(D, dtype=BF16),
    )
    meta = dict(P=P, E_pad=E_pad, n_chunks=n_chunks)
    return meta, per_core, shared


def _build(cfg, meta):
    global FAKE_SILU
    from concourse import bacc, tile, mybir
    import concourse.bass as bass

    nc = bacc.Bacc("TRN2", target_bir_lowering=False, debug=False,
                   num_devices=N_CORES)
    f32, i32, i16 = mybir.dt.float32, mybir.dt.int32, mybir.dt.int16
    bf16 = mybir.dt.bfloat16
    AF = mybir.ActivationFunctionType
    NSH, NSHP = cfg.NSH, cfg.NSHP
    AGOFF = [0]
    for g in range(cfg.NAG):
        AGOFF.append(AGOFF[-1] + N_CORES * (cfg.AGB[g + 1] - cfg.AGB[g]))
    P = meta["P"]
    n_chunks = meta["n_chunks"]
    E_pad = meta["E_pad"]
    TCH_MAX = int(max(P)) // 128

    srcg = nc.dram_tensor("srcg", [128, n_chunks], i32, kind="ExternalInput")
    apg = nc.dram_tensor("apg", [128, 8 * n_chunks], i16,
                         kind="ExternalInput")
    S_d = nc.dram_tensor("S", [128, TN * n_chunks], bf16,
                         kind="ExternalInput")
    srhs = nc.dram_tensor("srhs", [5, E_pad], bf16, kind="ExternalInput")
    feeds = nc.dram_tensor("feeds", [E_pad * 128], bf16,
                           kind="ExternalInput")
    feedt = nc.dram_tensor("feedt", [E_pad * 128], bf16,
                           kind="ExternalInput")
    hsh0T = nc.dram_tensor("hsh0T", [D, NSHP], f32, kind="ExternalInput")
    nomsg = nc.dram_tensor("nomsg", [1, NSHP], bf16, kind="ExternalInput")
    wnames = dict(W1srcT=[D, D], W1tgtT=[D, D], W1staT=[5, D],
                  b1col=[D, 1], W2T=[D, D], b2col=[D, 1],
                  WihT=[D, 3 * D], WhhT=[D, 3 * D], brzcol=[D, 2],
                  bnhcol=[D, 1], bnicol=[D, 1], ones1=[1, D], eye=[D, D])
    wbf = {"W1srcT", "W1tgtT", "W1staT", "W2T", "WihT", "WhhT", "ones1",
           "eye"}
    wt = {k: nc.dram_tensor(k, sh, bf16 if k in wbf else f32,
                            kind="ExternalInput")
          for k, sh in wnames.items()}
    h_outT = nc.dram_tensor("h_outT", [D, NSHP], f32, kind="ExternalOutput")
    h1rm = nc.dram_tensor("h1rm", [NSHP, D], bf16)
    h1full = nc.dram_tensor("h1full", [cfg.N, D], bf16, addr_space="Shared")

    with tile.TileContext(nc) as tc:
        with (
            tc.tile_pool(name="const", bufs=1) as cpool,
            tc.tile_pool(name="stream", bufs=4) as spool,
            tc.tile_pool(name="sfeed", bufs=5) as fpool,
            tc.tile_pool(name="work", bufs=6) as wpool,
            tc.tile_pool(name="gru", bufs=2) as upool,
            tc.tile_pool(name="pt", bufs=2, space="PSUM") as pt,
            tc.tile_pool(name="pg", bufs=4, space="PSUM") as pg,
            tc.tile_pool(name="pu", bufs=2, space="PSUM") as pu,
        ):
            w = {}
            for k, sh in wnames.items():
                w[k] = cpool.tile(sh, bf16 if k in wbf else f32,
                                  tag=k, name=f"w_{k}")
                nc.sync.dma_start(out=w[k][:, :], in_=wt[k][:, :])
            slab = cpool.tile([D, NSHP], f32, tag="slab")
            for t in range(cfg.NTILES):
                nc.sync.dma_start(out=slab[:, TN * t:TN * (t + 1)],
                                  in_=hsh0T[:, TN * t:TN * (t + 1)])
            srcg_sb = cpool.tile([128, n_chunks], i32, tag="srcg")
            nc.sync.dma_start(out=srcg_sb[:, :], in_=srcg[:, :])
            apg_sb = cpool.tile([128, 8 * n_chunks], i16, tag="apg")
            nc.sync.dma_start(out=apg_sb[:, :], in_=apg[:, :])

            for p in range(N_PASSES):
                ch0 = 0
                next_g = 0
                for t in range(cfg.NTILES):
                    tch = int(P[t]) // 128
                    # per-tile streamed structure (ACT HWDGE queue)
                    srh_t = spool.tile([5, 128 * TCH_MAX], bf16, tag="srh")
                    nc.scalar.dma_start(
                        out=srh_t[:, :128 * tch],
                        in_=srhs[:, 128 * ch0:128 * (ch0 + tch)])
                    nm_t = spool.tile([1, TN], bf16, tag="nm")
                    nc.scalar.dma_start(out=nm_t[:, :],
                                        in_=nomsg[:, TN * t:TN * (t + 1)])
                    aggT = pg.tile([128, TN], f32, tag="pg",
                                   name=f"agg{p}_{t}")
                    for b0 in range(0, tch, 4):
                        bw = min(4, tch - b0)
                        cb = ch0 + b0
                        S_t = spool.tile([128, TN * 4], bf16, tag="S")
                        nc.scalar.dma_start(
                            out=S_t[:, :TN * bw],
                            in_=S_d[:, TN * cb:TN * (cb + bw)])
                        sT = fpool.tile([128, 512], bf16, tag="sT")
                        tT = fpool.tile([128, 512], bf16, tag="tT")
                        if p == 0:
                            nc.sync.dma_start(
                                out=sT[:, :128 * bw]
                                .rearrange("p (a e) -> p a e", e=128),
                                in_=feeds[cb * 128 * 128:
                                          (cb + bw) * 128 * 128]
                                .rearrange("(a p e) -> p a e", p=128, e=128))
                            nc.sync.dma_start(
                                out=tT[:, :128 * bw]
                                .rearrange("p (a e) -> p a e", e=128),
                                in_=feedt[cb * 128 * 128:
                                          (cb + bw) * 128 * 128]
                                .rearrange("(a p e) -> p a e", p=128, e=128))
                        else:
                            # src: per-chunk indirect gathers + bf16 T
                            gsr = fpool.tile([128, 512], bf16, tag="gsr")
                            for a in range(bw):
                                nc.gpsimd.indirect_dma_start(
                                    out=gsr[:, 128 * a:128 * (a + 1)],
                                    out_offset=None,
                                    in_=h1full[:, :],
                                    in_offset=bass.IndirectOffsetOnAxis(
                                        ap=srcg_sb[:, cb + a:cb + a + 1],
                                        axis=0))
                            ps = pt.tile([128, 512], bf16, tag="ptb",
                                         name=f"ps{p}_{cb}")
                            for a in range(bw):
                                nc.tensor.transpose(
                                    ps[:, 128 * a:128 * (a + 1)],
                                    gsr[:, 128 * a:128 * (a + 1)],
                                    w["eye"][:, :])
                            nc.vector.tensor_copy(sT[:, :128 * bw],
                                                  ps[:, :128 * bw])
                            # tgt: one batched gpsimd gather from f32 slab
                            tTf = fpool.tile([128, 512], f32, tag="tTf")
                            nc.gpsimd.ap_gather(
                                tTf[:, :128 * bw],
                                slab[:, TN * t:TN * (t + 1)],
                                apg_sb[:, 8 * cb:8 * (cb + bw)],
                                channels=128, num_elems=TN, d=1,
                                num_idxs=128 * bw)
                            nc.vector.tensor_copy(tT[:, :128 * bw],
                                                  tTf[:, :128 * bw])
                        # layer 1 (const stationaries)
                        y1 = pg.tile([128, 512], f32, tag="pg",
                                     name=f"y1_{p}_{cb}")
                        nc.tensor.matmul(y1[:, :128 * bw], w["W1srcT"][:, :],
                                         sT[:, :128 * bw],
                                         start=True, stop=False)
                        nc.tensor.matmul(y1[:, :128 * bw], w["W1tgtT"][:, :],
                                         tT[:, :128 * bw],
                                         start=False, stop=False)
                        nc.tensor.matmul(
                            y1[:, :128 * bw], w["W1staT"][:, :],
                            srh_t[:, 128 * b0:128 * (b0 + bw)],
                            start=False, stop=True)
                        y1s = wpool.tile([128, 512], bf16, tag="y1s")
                        if FAKE_SILU:
                            zb = wpool.tile([128, 512], f32, tag="zb")
                            nc.scalar.activation(zb[:, :128 * bw],
                                                 y1[:, :128 * bw],
                                                 AF.Identity,
                                                 bias=w["b1col"][:, 0:1])
                            sg = wpool.tile([128, 512], f32, tag="sg")
                            nc.scalar.activation(sg[:, :128 * bw],
                                                 y1[:, :128 * bw],
                                                 AF.Sigmoid,
                                                 bias=w["b1col"][:, 0:1])
                            nc.vector.tensor_mul(y1s[:, :128 * bw],
                                                 zb[:, :128 * bw],
                                                 sg[:, :128 * bw])
                        else:
                            nc.scalar.activation(y1s[:, :128 * bw],
                                                 y1[:, :128 * bw], AF.Silu,
                                                 bias=w["b1col"][:, 0:1])
                        # layer 2 per chunk; one grouped cast
                        y2 = pg.tile([128, 512], f32, tag="pg",
                                     name=f"y2_{p}_{cb}")
                        for a in range(bw):
                            nc.tensor.matmul(
                                y2[:, 128 * a:128 * (a + 1)],
                                y1s[:, 128 * a:128 * (a + 1)],
                                w["W2T"][:, :], start=True, stop=True)
                        mp = wpool.tile([128, 512], bf16, tag="mp")
                        nc.vector.tensor_copy(mp[:, :128 * bw],
                                              y2[:, :128 * bw])
                        for a in range(bw):
                            nc.tensor.matmul(
                                aggT[:, :], mp[:, 128 * a:128 * (a + 1)],
                                S_t[:, TN * a:TN * (a + 1)],
                                start=(b0 == 0 and a == 0),
                                stop=(b0 + 4 >= tch and a == bw - 1))
                    ch0 += tch
                    # ---- GRU for this node tile (T layout, all-tanh)
                    cl, chh = TN * t, TN * (t + 1)
                    xT = upool.tile([128, TN], bf16, tag="xT")
                    nc.scalar.activation(xT[:, :], aggT[:, :], AF.Identity,
                                         bias=w["b2col"][:, 0:1])
                    hTb = upool.tile([128, TN], bf16, tag="hTb")
                    nc.vector.tensor_copy(hTb[:, :], slab[:, cl:chh])
                    pr = pu.tile([128, TN], f32, tag="pu", name=f"pr{p}_{t}")
                    pz = pu.tile([128, TN], f32, tag="pu", name=f"pz{p}_{t}")
                    nc.tensor.matmul(pr[:, :], w["WihT"][:, 0:D], xT[:, :],
                                     start=True, stop=False)
                    nc.tensor.matmul(pr[:, :], w["WhhT"][:, 0:D], hTb[:, :],
                                     start=False, stop=True)
                    nc.tensor.matmul(pz[:, :], w["WihT"][:, D:2 * D],
                                     xT[:, :], start=True, stop=False)
                    nc.tensor.matmul(pz[:, :], w["WhhT"][:, D:2 * D],
                                     hTb[:, :], start=False, stop=False)
                    nc.tensor.matmul(pz[:, :], w["ones1"][:, :],
                                     nm_t[:, :], start=False, stop=True)
                    t_r = upool.tile([128, TN], bf16, tag="t_r")
                    nc.scalar.activation(t_r[:, :], pr[:, :], AF.Tanh,
                                         bias=w["brzcol"][:, 0:1], scale=0.5)
                    t_z = upool.tile([128, TN], bf16, tag="t_z")
                    nc.scalar.activation(t_z[:, :], pz[:, :], AF.Tanh,
                                         bias=w["brzcol"][:, 1:2], scale=0.5)
                    pni = pu.tile([128, TN], f32, tag="pu", name=f"pi{p}_{t}")
                    pnh = pu.tile([128, TN], f32, tag="pu", name=f"ph{p}_{t}")
                    nc.tensor.matmul(pni[:, :], w["WihT"][:, 2 * D:3 * D],
                                     xT[:, :], start=True, stop=True)
                    nc.tensor.matmul(pnh[:, :], w["WhhT"][:, 2 * D:3 * D],
                                     hTb[:, :], start=True, stop=True)
                    ghn = upool.tile([128, TN], bf16, tag="ghn")
                    nc.vector.tensor_scalar(
                        out=ghn[:, :], in0=pnh[:, :],
                        scalar1=w["bnhcol"][:, 0:1], scalar2=None,
                        op0=mybir.AluOpType.add)
                    m1 = upool.tile([128, TN], bf16, tag="m1")
                    nc.vector.tensor_mul(m1[:, :], t_r[:, :], ghn[:, :])
                    m2 = upool.tile([128, TN], bf16, tag="m2")
                    nc.vector.tensor_add(m2[:, :], m1[:, :], ghn[:, :])
                    m4 = upool.tile([128, TN], bf16, tag="m4")
                    nc.vector.tensor_add(m4[:, :], m2[:, :], pni[:, :])
                    n_s = upool.tile([128, TN], bf16, tag="n_s")
                    nc.scalar.activation(n_s[:, :], m4[:, :], AF.Tanh,
                                         bias=w["bnicol"][:, 0:1], scale=0.5)
                    d_s = upool.tile([128, TN], bf16, tag="m1")
                    nc.vector.tensor_sub(d_s[:, :], hTb[:, :], n_s[:, :])
                    e_s = upool.tile([128, TN], bf16, tag="m2")
                    nc.vector.tensor_mul(e_s[:, :], t_z[:, :], d_s[:, :])
                    f_s = upool.tile([128, TN], bf16, tag="m4")
                    nc.vector.tensor_add(f_s[:, :], n_s[:, :], hTb[:, :])
                    g_s = upool.tile([128, TN], bf16, tag="g_s")
                    nc.vector.tensor_add(g_s[:, :], e_s[:, :], f_s[:, :])
                    nc.vector.tensor_scalar(
                        out=slab[:, cl:chh], in0=g_s[:, :],
                        scalar1=0.5, scalar2=None,
                        op0=mybir.AluOpType.mult)
                    if p == 0:
                        # bf16 rows for the AllGather
                        hn_bf = upool.tile([128, TN], bf16, tag="hn_bf")
                        nc.vector.tensor_scalar(
                            out=hn_bf[:, :], in0=g_s[:, :],
                            scalar1=0.5, scalar2=None,
                            op0=mybir.AluOpType.mult)
                        pb = pt.tile([128, 512], bf16, tag="ptb",
                                     name=f"pb{p}_{t}")
                        for a in range(4):
                            nc.tensor.transpose(
                                pb[:, 128 * a:128 * (a + 1)],
                                hn_bf[:, 128 * a:128 * (a + 1)],
                                w["eye"][:, :])
                        hrows = upool.tile([128, TN], bf16, tag="hrows")
                        nc.vector.tensor_copy(hrows[:, :], pb[:, :])
                        nc.sync.dma_start(
                            out=h1rm[cl:chh, :].rearrange(
                                "(a q) d -> q a d", q=128),
                            in_=hrows[:, :].rearrange(
                                "q (a d) -> q a d", d=128))
                        while (next_g < cfg.NAG
                               and (t + 1) * TN >= cfg.AGB[next_g + 1]):
                            g = next_g
                            r0, r1 = cfg.AGB[g], cfg.AGB[g + 1]
                            o0 = AGOFF[g]
                            nc.gpsimd.collective_compute(
                                "AllGather", mybir.AluOpType.bypass,
                                replica_groups=[list(range(N_CORES))],
                                ins=[h1rm[r0:r1, :]],
                                outs=[h1full[o0:o0 + N_CORES * (r1 - r0),
                                             :]])
                            next_g += 1
                    else:
                        nc.sync.dma_start(out=h_outT[:, cl:chh],
                                          in_=slab[:, cl:chh])
    nc.compile()
    return nc


def build_and_run(inputs, cfg=None, sim=False, trace=False, tmpdir=None):
    global FAKE_SILU
    cfg = cfg or _Cfg()
    meta, per_core, shared = _plan(cfg, inputs)
    FAKE_SILU = bool(sim)
    nc = _build(cfg, meta)
    maps = []
    for c in range(N_CORES):
        m = {k: np.ascontiguousarray(v) for k, v in per_core[c].items()}
        m.update({k: np.ascontiguousarray(v) for k, v in shared.items()})
        maps.append(m)
    if sim:
        from concourse.bass_interp import MultiCoreSim
        ms = MultiCoreSim(nc, num_cores=N_CORES, trace=False)
        for c in range(N_CORES):
            for k, v in maps[c].items():
                ms.cores[c].tensor(k)[:] = v
        ms.simulate(check_with_hw=False)
        shards = [np.array(ms.cores[c].tensor("h_outT"))[:, :cfg.NSH].T
                  for c in range(N_CORES)]
        return np.concatenate(shards, axis=0), None
    from concourse import bass_utils
    res = bass_utils.run_bass_kernel_spmd(
        nc, maps, list(range(N_CORES)), trace=trace, tmpdir=tmpdir)
    shards = [res.results[c]["h_outT"][:, :cfg.NSH].T for c in range(N_CORES)]
    return np.concatenate(shards, axis=0), res


def kernel(**inputs):
    out, _ = build_and_run(inputs)
    return np.ascontiguousarray(out).astype(np.float32)
